# revision 1
# baseline (speedup 1.0000x reference)
"""Trainium2 Bass kernel for CurlVectorField.

curl(psi) where psi = W3 tanh(W2 tanh(W1 x + b1) + b2) + b3, x in R^3,
N = 524288 points. Data-parallel over 8 NeuronCores.

Math (per point, h1 = tanh(W1x+b1), s = h1^2, D1 = 1-s, h2 = tanh(W2h1+b2),
D2 = 1-h2^2):
  J[i,j] = sum_hk W3[i,h] D2[h] W2[h,k] D1[k] W1[k,j]
  curl   = (J21-J12, J02-J20, J10-J01)
Fold weights: M_j = W2 * W1[:,j] (col-scaled), B_c = antisym combos of
W3-rows x M_j, cst_c[h] = sum_k B_c[h,k]:
  curl_c = sum_h D2[h]*cst_c[h] - sum_h D2[h]*(B_c @ s)[h]

Layout: TWO points per column ("2-pack") — H=64 so [pt_even; pt_odd]
features fill all 128 partitions. Per iteration (1024 points, 512 cols):
  psum1 = blockdiag(W1T,W1T).T @ xt6     (128,512) h1pre   [f32r]
  S     = tanh(psum1 + b1d)              ACT -> f32r
  S2    = S*S                            ACT (Square)
  psum2 = blockdiag(W2T,W2T).T @ S       (128,512) h2pre
  T2    = tanh(psum2 + b2d)              ACT -> f32
  D2    = 1 - T2*T2                      Pool (mult + 1-x) -> f32r
  psumY_c = blockdiag(-Bc,-Bc).T @ S2    (128,512) x3, 3-buf pool
  V[:,c,:] = psumY_c * D2                DVE (psum evac + weight) -> f32r
  psum5q: 3 iterations share one bank via reverse strips (rows 64:70,
  32:38, 0:6) with zero-padded stationaries (f32r matmuls must write
  PSUM at partition 0); 4 accumulating matmuls per iteration:
      gsel_c.T @ V_c  (c=0,1,2)  +  gcst.T @ D2
  every 3 iters: ACT copy psum5q -> SBUF, DMA 6-row strips to yt6
Host packs x as (6, NSH/2) [even xyz; odd xyz] and unpacks yt6 (6, NSH/2).
"""

import os
import sys
from contextlib import ExitStack

import numpy as np

sys.path.insert(0, "/opt/trn_rl_repo")

import concourse.bass as bass
import concourse.bacc as bacc
import concourse.tile as tile
from concourse import mybir
from concourse.bass_utils import run_bass_kernel_spmd

N_CORES = 8
NPTS = 524288
NSH = NPTS // N_CORES          # 65536 points per core
NSH2 = NSH // 2                # 32768 columns per core
H = 64
TILE_N = 512
NT = NSH2 // TILE_N            # 64 iterations per core

F32 = mybir.dt.float32
F32R = mybir.dt.float32r
BF16 = mybir.dt.bfloat16


def _build_program():
    nc = bacc.Bacc(
        "TRN2",
        target_bir_lowering=False,
        debug=False,
        num_devices=N_CORES,
    )

    xt6 = nc.dram_tensor("xt6", [6, NSH2], F32R, kind="ExternalInput").ap()
    w1bd = nc.dram_tensor("w1bd", [6, 128], F32R, kind="ExternalInput").ap()
    b1d = nc.dram_tensor("b1d", [128, 1], F32, kind="ExternalInput").ap()
    w2bd = nc.dram_tensor("w2bd", [128, 128], F32R, kind="ExternalInput").ap()
    b2d = nc.dram_tensor("b2d", [128, 1], F32, kind="ExternalInput").ap()
    bB = nc.dram_tensor("bB", [128, 3, 128], F32R, kind="ExternalInput").ap()
    gsel = nc.dram_tensor("gsel", [128, 3, 114], F32R, kind="ExternalInput").ap()
    gcst = nc.dram_tensor("gcst", [128, 114], F32R, kind="ExternalInput").ap()
    yt6 = nc.dram_tensor("yt6", [6, NSH2], F32, kind="ExternalOutput").ap()

    with tile.TileContext(nc) as tc, ExitStack() as ctx:
        consts = ctx.enter_context(tc.tile_pool(name="consts", bufs=1))
        xin = ctx.enter_context(tc.tile_pool(name="xin", bufs=3))
        sb = ctx.enter_context(tc.tile_pool(name="sb", bufs=4))
        outp = ctx.enter_context(tc.tile_pool(name="outp", bufs=2))
        pp1 = ctx.enter_context(tc.tile_pool(name="pp1", bufs=2, space="PSUM"))
        pp2 = ctx.enter_context(tc.tile_pool(name="pp2", bufs=2, space="PSUM"))
        ppy = ctx.enter_context(tc.tile_pool(name="ppy", bufs=3, space="PSUM"))
        pp5 = ctx.enter_context(tc.tile_pool(name="pp5", bufs=1, space="PSUM"))

        w1bd_s = consts.tile([6, 128], F32R)
        b1d_s = consts.tile([128, 1], F32)
        w2bd_s = consts.tile([128, 128], F32R)
        b2d_s = consts.tile([128, 1], F32)
        bB_s = consts.tile([128, 3, 128], F32R)
        gsel_s = consts.tile([128, 3, 114], F32R)
        gcst_s = consts.tile([128, 114], F32R)
        for dst, src in (
            (w1bd_s, w1bd), (b1d_s, b1d), (w2bd_s, w2bd), (b2d_s, b2d),
            (bB_s, bB), (gsel_s, gsel), (gcst_s, gcst),
        ):
            nc.sync.dma_start(out=dst, in_=src)

        psum5q = None
        pend = []
        for t in range(NT):
            sl = slice(t * TILE_N, (t + 1) * TILE_N)

            xt_t = xin.tile([6, TILE_N], F32R)
            nc.sync.dma_start(out=xt_t, in_=xt6[:, sl])

            psum1 = pp1.tile([128, TILE_N], F32, tag="psum1")
            nc.tensor.matmul(psum1, w1bd_s[:, :], xt_t[:, :],
                             start=True, stop=True)

            S = sb.tile([128, TILE_N], F32R, tag="S")
            nc.scalar.activation(S[:, :], psum1[:, :],
                                 mybir.ActivationFunctionType.Tanh,
                                 bias=b1d_s[:, :])
            S2 = sb.tile([128, TILE_N], F32R, tag="S2")
            nc.scalar.activation(S2[:, :], S[:, :], mybir.ActivationFunctionType.Square)

            psum2 = pp2.tile([128, TILE_N], F32, tag="psum2")
            nc.tensor.matmul(psum2, w2bd_s[:, :], S[:, :],
                             start=True, stop=True)

            T2 = sb.tile([128, TILE_N], F32, tag="T2")
            nc.scalar.activation(T2[:, :], psum2[:, :],
                                 mybir.ActivationFunctionType.Tanh,
                                 bias=b2d_s[:, :])

            D2 = sb.tile([128, TILE_N], F32R, tag="D2")
            nc.gpsimd.tensor_mul(D2[:, :], T2[:, :], T2[:, :])
            nc.gpsimd.tensor_scalar(D2[:, :], D2[:, :], -1.0, 1.0,
                                    mybir.AluOpType.mult,
                                    mybir.AluOpType.add)

            V = sb.tile([128, 3, TILE_N], F32R, tag="V")
            for c in range(3):
                psumY = ppy.tile([128, TILE_N], F32, tag="psumY")
                nc.tensor.matmul(psumY[:, :], bB_s[:, c, :], S2[:, :],
                                 start=True, stop=True)
                nc.vector.tensor_mul(V[:, c, :], psumY[:, :], D2[:, :])

            # reverse-strip packing: group iter 0 -> psum rows 64:70
            # (M=70, start=True clears rows 0:70), iter 1 -> rows 32:38
            # (M=38, accumulate onto zeros), iter 2 -> rows 0:6 (M=6).
            # Zero-padded stationaries keep every matmul dst at partition 0
            # (f32r matmuls cannot write PSUM at a partition offset).
            if not pend:
                psum5q = pp5.tile([128, TILE_N], F32, tag="psum5q")
            r = len(pend)
            gslc = (slice(44, 114), slice(6, 44), slice(0, 6))[r]
            m = (70, 38, 6)[r]
            rowbase = (64, 32, 0)[r]
            last = (r == 2) or (t == NT - 1)
            for c in range(3):
                nc.tensor.matmul(psum5q[0:m, :], gsel_s[:, c, gslc],
                                 V[:, c, :],
                                 start=(r == 0 and c == 0), stop=False,
                                 skip_group_check=True)
            nc.tensor.matmul(psum5q[0:m, :], gcst_s[:, gslc], D2[:, :],
                             start=False, stop=last, skip_group_check=True)
            pend.append((rowbase, sl))

            if last:
                yq = outp.tile([128, TILE_N], F32, tag="yq")
                nc.scalar.copy(yq[:, :], psum5q[:, :])
                for (rb, ssl) in pend:
                    nc.sync.dma_start(out=yt6[:, ssl],
                                      in_=yq[rb:rb + 6, :])
                pend = []

    nc.compile()
    return nc


_NC_CACHE = None


def _get_program():
    global _NC_CACHE
    if _NC_CACHE is None:
        _NC_CACHE = _build_program()
    return _NC_CACHE


def _host_weights(W1, b1, W2, b2, W3):
    W1 = np.asarray(W1, np.float32)
    W2 = np.asarray(W2, np.float32)
    W3 = np.asarray(W3, np.float32)
    b1 = np.asarray(b1, np.float32)
    b2 = np.asarray(b2, np.float32)
    M = np.einsum("hk,kj->jhk", W2, W1)          # M_j = W2 * W1[:,j]
    B = np.stack([
        W3[2][:, None] * M[1] - W3[1][:, None] * M[2],
        W3[0][:, None] * M[2] - W3[2][:, None] * M[0],
        W3[1][:, None] * M[0] - W3[0][:, None] * M[1],
    ]).astype(np.float32)                         # (3, H, H)
    cst = B.sum(axis=2)                           # (3, H)

    Z = np.zeros((64, 64), np.float32)
    bd = lambda A: np.block([[A, Z], [Z, A]]).astype(np.float32)

    w1bd = np.zeros((6, 128), np.float32)
    w1bd[0:3, 0:64] = W1.T
    w1bd[3:6, 64:128] = W1.T

    gsel6 = np.zeros((3, 128, 6), np.float32)
    for c in range(3):
        gsel6[c, 0:64, c] = 1.0
        gsel6[c, 64:128, 3 + c] = 1.0
    gcst6 = np.zeros((128, 6), np.float32)
    for c in range(3):
        gcst6[0:64, c] = cst[c]
        gcst6[64:128, 3 + c] = cst[c]
    # packed reverse-strip variants: [strip2 M=70 | strip1 M=38 | strip0 M=6]
    gsel = np.zeros((3, 128, 114), np.float32)
    gcst = np.zeros((128, 114), np.float32)
    gsel[:, :, 0:6] = gsel6          # strip0 rows 0:6
    gcst[:, 0:6] = gcst6
    gsel[:, :, 6 + 32:44] = gsel6    # strip1 rows 32:38
    gcst[:, 6 + 32:44] = gcst6
    gsel[:, :, 44 + 64:114] = gsel6  # strip2 rows 64:70
    gcst[:, 44 + 64:114] = gcst6

    c_ = np.ascontiguousarray
    return {
        "w1bd": c_(w1bd),
        "b1d": c_(np.concatenate([b1, b1])[:, None]),
        "w2bd": bd(W2.T),
        "b2d": c_(np.concatenate([b2, b2])[:, None]),
        "bB": c_(np.stack([bd(-B[c].T) for c in range(3)], axis=1)),
        "gsel": np.ascontiguousarray(gsel.transpose(1, 0, 2)),
        "gcst": gcst,
    }


def kernel(x, W1, b1, W2, b2, W3, b3, _want_trace=False):
    x = np.asarray(x, np.float32)
    wts = _host_weights(W1, b1, W2, b2, W3)

    in_maps = []
    for ci in range(N_CORES):
        xs = x[ci * NSH:(ci + 1) * NSH]                       # (NSH, 3)
        xt6 = np.ascontiguousarray(
            xs.reshape(NSH2, 2, 3).transpose(1, 2, 0).reshape(6, NSH2))
        m = {"xt6": xt6}
        m.update(wts)
        in_maps.append(m)

    nc = _get_program()
    res = None
    for attempt in range(3):
        try:
            res = run_bass_kernel_spmd(nc, in_maps, list(range(N_CORES)),
                                       trace=_want_trace)
            break
        except Exception as e:
            # Axon-tunneled NeuronCores occasionally report a transient
            # NRT_EXEC_UNIT_UNRECOVERABLE; a retry on the same devices
            # consistently succeeds.
            if attempt == 2 or "UNRECOVERABLE" not in str(e).upper():
                raise
            import time
            time.sleep(10)
    outs = []
    for ci in range(N_CORES):
        yt6 = res.results[ci]["yt6"]                          # (6, NSH2)
        y = yt6.reshape(2, 3, NSH2).transpose(2, 0, 1).reshape(NSH, 3)
        outs.append(y)
    out = np.ascontiguousarray(np.concatenate(outs, axis=0)).astype(np.float32)
    if _want_trace:
        return out, res
    return out

